# revision 5
# baseline (speedup 1.0000x reference)
"""Trainium2 Bass kernel for nn_Neighbor_PairConLoss.

Computes, for pos = concat(feat1[i1], feat2[i1]) and neg = concat(feat1[i2], feat2[i2])
(both [8192, 256]):
    Ng = exp(pos @ neg.T / T).sum(-1);  Pg = exp(pos @ pos.T / T).sum(-1)
    loss = mean(-log(Pg / (Ng + Pg)))

Strategy: host-side gather + transpose (cheap, no HW time), then shard the 8192
rows of pos across 8 NeuronCores. Each core holds full negT/posT ([256, 8192],
feature-major) in SBUF plus its own 1024-row stationary slice, runs fp32r
matmuls into PSUM (N=512 per bank, groups of 4 banks), and fuses exp + row-sum
on the scalar engine via activation(Exp, scale=1/T, accum_out=...). Raw group
sums [128, 64] stream back per core; the final (tiny) reduction + log happens
on host in float32, matching the reference's IEEE semantics.
"""

import numpy as np

N_FEAT = 16384
D = 256
M_IDX = 4096
ROWS = 2 * M_IDX          # 8192 rows in pos/neg
INV_T = 20.0              # 1 / 0.05
N_CORES = 8
RPC = ROWS // N_CORES     # 1024 rows per core
RT = RPC // 128           # 8 row-tiles per core
KCH = D // 128            # 2 contraction chunks
CT = 512                  # matmul moving free dim (one PSUM bank)
GRP = 4 * CT              # 2048 columns per exp/accum group (4 banks)
NG = ROWS // GRP          # 4 groups per matrix per row-tile
ACC_COLS = RT * 2 * NG    # 64 accumulator columns per core

_PROG = None  # cached (nc, meta)


def _build_program():
    import concourse.bass as bass
    import concourse.tile as tile
    from concourse import bacc, mybir

    f32 = mybir.dt.float32
    f32r = mybir.dt.float32r
    EXP = mybir.ActivationFunctionType.Exp

    nc = bacc.Bacc("TRN2", target_bir_lowering=False, debug=False)
    posT = nc.dram_tensor("posT", [KCH, 128, ROWS], f32r, kind="ExternalInput").ap()
    negT = nc.dram_tensor("negT", [KCH, 128, ROWS], f32r, kind="ExternalInput").ap()
    myT = nc.dram_tensor("myT", [KCH, 128, RPC], f32r, kind="ExternalInput").ap()
    out = nc.dram_tensor("out", [128, ACC_COLS], f32, kind="ExternalOutput").ap()

    with tile.TileContext(nc) as tc:
        with tc.tile_pool(name="inp", bufs=1) as inp, \
             tc.tile_pool(name="ps", bufs=2, space="PSUM") as psp, \
             tc.tile_pool(name="scr", bufs=2) as scr, \
             tc.tile_pool(name="accp", bufs=1) as accp:

            # SBUF-resident inputs, chunked so matmuls only wait on the
            # chunks they read.
            my_ch = []
            for k in range(KCH):
                t = inp.tile([128, RPC], f32r, tag=f"my{k}")
                nc.sync.dma_start(t[:], myT[k])
                my_ch.append(t)
            neg_ch = [[None] * NG for _ in range(KCH)]
            pos_ch = [[None] * NG for _ in range(KCH)]
            for g in range(NG):
                for k in range(KCH):
                    t = inp.tile([128, GRP], f32r, tag=f"neg{k}_{g}")
                    nc.sync.dma_start(t[:], negT[k][:, g * GRP:(g + 1) * GRP])
                    neg_ch[k][g] = t
            for g in range(NG):
                for k in range(KCH):
                    t = inp.tile([128, GRP], f32r, tag=f"pos{k}_{g}")
                    nc.sync.dma_start(t[:], posT[k][:, g * GRP:(g + 1) * GRP])
                    pos_ch[k][g] = t

            acc = accp.tile([128, ACC_COLS], f32)

            # phase 0: Ng (vs neg), phase 1: Pg (vs pos). Ng first so compute
            # can start while posT still streams in.
            for phase, X_ch in ((0, neg_ch), (1, pos_ch)):
                for m in range(RT):
                    for g in range(NG):
                        ps = psp.tile([128, GRP], f32, tag="ps")
                        for k in range(KCH):
                            lhsT = my_ch[k][:, m * 128:(m + 1) * 128]
                            for n in range(4):
                                nc.tensor.matmul(
                                    ps[:, n * CT:(n + 1) * CT],
                                    lhsT,
                                    X_ch[k][g][:, n * CT:(n + 1) * CT],
                                    start=(k == 0),
                                    stop=(k == KCH - 1),
                                )
                        sc = scr.tile([128, GRP], f32, tag="sc")
                        col = m * 8 + phase * 4 + g
                        nc.scalar.activation(
                            sc[:], ps[:], EXP, scale=INV_T,
                            accum_out=acc[:, col:col + 1],
                        )
            nc.sync.dma_start(out[:], acc[:])

    nc.compile()
    return nc


def _get_program():
    global _PROG
    if _PROG is None:
        _PROG = _build_program()
    return _PROG


def _round_fp32r(x):
    """Round float32 to fp32r (1s + 8e + 11m, low 12 bits zero), nearest-even."""
    u = x.view(np.uint32)
    lsb = (u >> 12) & np.uint32(1)
    r = (u + np.uint32(0x7FF) + lsb) & np.uint32(0xFFFFF000)
    return r.view(np.float32)


def _prep_inputs(feat1, feat2, indices1, indices2):
    feat1 = np.asarray(feat1, dtype=np.float32)
    feat2 = np.asarray(feat2, dtype=np.float32)
    i1 = np.asarray(indices1).astype(np.int64)
    i2 = np.asarray(indices2).astype(np.int64)
    pos = np.concatenate([feat1[i1], feat2[i1]], axis=0)          # [8192, 256]
    neg = np.concatenate([feat1[i2], feat2[i2]], axis=0)
    posT = _round_fp32r(np.ascontiguousarray(pos.T)).reshape(KCH, 128, ROWS)
    negT = _round_fp32r(np.ascontiguousarray(neg.T)).reshape(KCH, 128, ROWS)
    in_maps = []
    for c in range(N_CORES):
        in_maps.append({
            "posT": posT,
            "negT": negT,
            "myT": np.ascontiguousarray(posT[:, :, c * RPC:(c + 1) * RPC]),
        })
    return in_maps


def _assemble_loss(results):
    # results[c]["out"] is [128, 64]: col = m*8 + phase*4 + g
    ng = np.empty(ROWS, dtype=np.float32)
    pg = np.empty(ROWS, dtype=np.float32)
    for c in range(N_CORES):
        o = results[c]["out"]  # [128, ACC_COLS]
        for m in range(RT):
            base = c * RPC + m * 128
            blk = o[:, m * 8:m * 8 + 8]                  # [128, 8]
            ng[base:base + 128] = blk[:, 0:4].sum(axis=1, dtype=np.float32)
            pg[base:base + 128] = blk[:, 4:8].sum(axis=1, dtype=np.float32)
    with np.errstate(divide="ignore", invalid="ignore", over="ignore"):
        ratio = pg / (ng + pg)
        loss = -np.log(ratio)
    return np.float32(np.mean(loss, dtype=np.float32))


def _run_on_hw(in_maps):
    from concourse.bass_utils import run_bass_kernel_spmd
    nc = _get_program()
    res = run_bass_kernel_spmd(nc, in_maps, list(range(N_CORES)))
    return res.results


def kernel(feat1, feat2, indices1, indices2):
    in_maps = _prep_inputs(feat1, feat2, indices1, indices2)
    results = _run_on_hw(in_maps)
    return _assemble_loss(results)


# revision 20
# speedup vs baseline: 1.0359x; 1.0359x over previous
"""Trainium2 Bass kernel for nn_Neighbor_PairConLoss.

Computes, for pos = concat(feat1[i1], feat2[i1]) and neg = concat(feat1[i2], feat2[i2])
(both [8192, 256]):
    Ng = exp(pos @ neg.T / T).sum(-1);  Pg = exp(pos @ pos.T / T).sum(-1)
    loss = mean(-log(Pg / (Ng + Pg)))

Strategy: host-side gather + transpose (cheap, no HW time), then shard the 8192
rows of pos across 8 NeuronCores. Each core holds full negT/posT ([256, 8192],
feature-major) in SBUF plus its own 1024-row stationary slice, runs fp32r
matmuls into PSUM (N=512 per bank, groups of 4 banks), and fuses exp + row-sum
on the scalar engine via activation(Exp, scale=1/T, accum_out=...). Raw group
sums [128, 64] stream back per core; the final (tiny) reduction + log happens
on host in float32, matching the reference's IEEE semantics.
"""

import numpy as np

N_FEAT = 16384
D = 256
M_IDX = 4096
ROWS = 2 * M_IDX          # 8192 rows in pos/neg
INV_T = 20.0              # 1 / 0.05
N_CORES = 8
RPC = ROWS // N_CORES     # 1024 rows per core
RT = RPC // 128           # 8 row-tiles per core
KCH = D // 128            # 2 contraction chunks
CT = 512                  # matmul moving free dim (one PSUM bank)
GRP = 4 * CT              # 2048 columns per exp/accum group (4 banks)
NG = ROWS // GRP          # 4 groups per matrix per row-tile
# accumulator column layout: col = phase*32 + g*8 + m, plus 3 extra
# columns (64..66) holding the n=1..3 sub-sums of the split first group
# (phase=0, g=0, m=0, whose n=0 sub-sum lives in its normal col 0).
ACC_COLS = RT * 2 * NG + 3

_PROG = None  # cached (nc, meta)


def _build_program(repeats=1):
    import concourse.bass as bass
    import concourse.tile as tile
    from concourse import bacc, mybir

    f32 = mybir.dt.float32
    f32r = mybir.dt.float32r
    EXP = mybir.ActivationFunctionType.Exp

    nc = bacc.Bacc("TRN2", target_bir_lowering=False, debug=False)
    posT = nc.dram_tensor("posT", [KCH, 128, ROWS], f32r, kind="ExternalInput").ap()
    negT = nc.dram_tensor("negT", [KCH, 128, ROWS], f32r, kind="ExternalInput").ap()
    # my stationary slices, repacked host-side as [row-tile, partition, k*128+j]
    myT = nc.dram_tensor("myT", [RT, 128, KCH * 128], f32r,
                         kind="ExternalInput").ap()
    out = nc.dram_tensor("out", [128, ACC_COLS], f32, kind="ExternalOutput").ap()

    with tile.TileContext(nc) as tc:
        with tc.tile_pool(name="inp", bufs=1) as inp, \
             tc.tile_pool(name="ps", bufs=2, space="PSUM") as psp, \
             tc.tile_pool(name="scr", bufs=2) as scr, \
             tc.tile_pool(name="accp", bufs=1) as accp:

            # SBUF-resident inputs, chunked so matmuls only wait on the
            # chunks they read. DMA issue order = need order: the m=0
            # stationary slices and the first column-group gate the pipeline.
            my_t = [None] * RT
            neg_ch = [[None] * NG for _ in range(KCH)]
            pos_ch = [[None] * NG for _ in range(KCH)]

            def load_my(m):
                t = inp.tile([128, KCH * 128], f32r, tag=f"my_{m}")
                nc.sync.dma_start(t[:], myT[m])
                my_t[m] = t

            def my_ap(k, m):
                return my_t[m][:, k * 128:(k + 1) * 128]

            def load_grp(dst, src, g, name):
                for k in range(KCH):
                    t = inp.tile([128, GRP], f32r, tag=f"{name}{k}_{g}")
                    nc.sync.dma_start(t[:], src[k][:, g * GRP:(g + 1) * GRP])
                    dst[k][g] = t

            # First column-group of neg arrives in 512-col sub-chunks so the
            # pipeline starts as early as possible.
            neg_sub = [[None] * 4 for _ in range(KCH)]

            def load_neg0_sub(n):
                for k in range(KCH):
                    t = inp.tile([128, CT], f32r, tag=f"negs{k}_{n}")
                    nc.sync.dma_start(t[:], negT[k][:, n * CT:(n + 1) * CT])
                    neg_sub[k][n] = t

            load_my(0)
            for n in range(4):
                load_neg0_sub(n)
            for m in range(1, RT):
                load_my(m)
            for g in range(1, NG):
                load_grp(neg_ch, negT, g, "neg")
            for g in range(NG):
                load_grp(pos_ch, posT, g, "pos")

            acc = accp.tile([128, ACC_COLS], f32)

            # Warm the ACT exp table (~2.7us) under the initial DMAs.
            warm = scr.tile([128, GRP], f32, tag="sc")
            nc.scalar.activation(warm[:, 0:1], warm[:, 1:2], EXP, scale=0.0)

            # phase 0: Ng (vs neg), phase 1: Pg (vs pos). Ng first so compute
            # can start while posT still streams in. g outer / m inner: the
            # first 2MB column-group feeds 8 row-tiles of compute, hiding the
            # DMA stream-in of later groups.
            def rhs_tile(phase, g, k, n):
                if phase == 0 and g == 0:
                    return neg_sub[k][n][:]
                X_ch = neg_ch if phase == 0 else pos_ch
                return X_ch[k][g][:, n * CT:(n + 1) * CT]

            for rep in range(repeats):
              for phase in (0, 1):
                for g in range(NG):
                    for m in range(RT):
                        col = phase * 32 + g * 8 + m
                        ps = psp.tile([128, GRP], f32, tag="ps")
                        first = (rep == 0 and phase == 0 and g == 0 and m == 0)
                        if first:
                            # Split the very first group per 512-col bank so
                            # exp starts as soon as the first chunks land.
                            for n in range(4):
                                for k in range(KCH):
                                    nc.tensor.matmul(
                                        ps[:, n * CT:(n + 1) * CT],
                                        my_ap(k, m),
                                        rhs_tile(phase, g, k, n),
                                        start=(k == 0),
                                        stop=(k == KCH - 1),
                                    )
                                sc = scr.tile([128, GRP], f32, tag="sc")
                                subcol = col if n == 0 else 63 + n
                                nc.scalar.activation(
                                    sc[:, 0:CT], ps[:, n * CT:(n + 1) * CT],
                                    EXP, scale=INV_T,
                                    accum_out=acc[:, subcol:subcol + 1],
                                )
                            continue
                        for k in range(KCH):
                            lhsT = my_ap(k, m)
                            for n in range(4):
                                nc.tensor.matmul(
                                    ps[:, n * CT:(n + 1) * CT],
                                    lhsT,
                                    rhs_tile(phase, g, k, n),
                                    start=(k == 0),
                                    stop=(k == KCH - 1),
                                )
                        sc = scr.tile([128, GRP], f32, tag="sc")
                        nc.scalar.activation(
                            sc[:], ps[:], EXP, scale=INV_T,
                            accum_out=acc[:, col:col + 1],
                        )
                if phase == 0:
                    # Ng columns are final: stream them out mid-kernel.
                    nc.sync.dma_start(out[:, 0:32], acc[:, 0:32])
                    nc.sync.dma_start(out[:, 64:ACC_COLS], acc[:, 64:ACC_COLS])
            nc.sync.dma_start(out[:, 32:64], acc[:, 32:64])

    nc.compile()
    return nc


def _get_program():
    global _PROG
    if _PROG is None:
        _PROG = _build_program()
    return _PROG


def _round_fp32r(x):
    """Round float32 to fp32r (1s + 8e + 11m, low 12 bits zero), nearest-even."""
    u = x.view(np.uint32)
    lsb = (u >> 12) & np.uint32(1)
    r = (u + np.uint32(0x7FF) + lsb) & np.uint32(0xFFFFF000)
    return r.view(np.float32)


def _prep_inputs(feat1, feat2, indices1, indices2):
    feat1 = np.asarray(feat1, dtype=np.float32)
    feat2 = np.asarray(feat2, dtype=np.float32)
    i1 = np.asarray(indices1).astype(np.int64)
    i2 = np.asarray(indices2).astype(np.int64)
    pos = np.concatenate([feat1[i1], feat2[i1]], axis=0)          # [8192, 256]
    neg = np.concatenate([feat1[i2], feat2[i2]], axis=0)
    posT = _round_fp32r(np.ascontiguousarray(pos.T)).reshape(KCH, 128, ROWS)
    negT = _round_fp32r(np.ascontiguousarray(neg.T)).reshape(KCH, 128, ROWS)
    in_maps = []
    for c in range(N_CORES):
        # myT[m, p, k*128+j] = posT[k, p, c*RPC + m*128 + j]
        sl = posT[:, :, c * RPC:(c + 1) * RPC].reshape(KCH, 128, RT, 128)
        myT = np.ascontiguousarray(sl.transpose(2, 1, 0, 3)).reshape(
            RT, 128, KCH * 128)
        in_maps.append({"posT": posT, "negT": negT, "myT": myT})
    return in_maps


def _assemble_loss(results):
    # results[c]["out"] is [128, 67]: col = phase*32 + g*8 + m, and cols
    # 64..66 hold the n=1..3 sub-sums of (phase0, g0, m0).
    ng = np.empty(ROWS, dtype=np.float32)
    pg = np.empty(ROWS, dtype=np.float32)
    for c in range(N_CORES):
        o = results[c]["out"]  # [128, ACC_COLS]
        for m in range(RT):
            base = c * RPC + m * 128
            cols = [g * 8 + m for g in range(NG)]
            ng_sum = o[:, cols].sum(axis=1, dtype=np.float32)
            if m == 0:
                ng_sum = (ng_sum + o[:, 64:67].sum(axis=1, dtype=np.float32)
                          ).astype(np.float32)
            ng[base:base + 128] = ng_sum
            pg[base:base + 128] = o[:, [32 + x for x in cols]].sum(
                axis=1, dtype=np.float32)
    with np.errstate(divide="ignore", invalid="ignore", over="ignore"):
        ratio = pg / (ng + pg)
        loss = -np.log(ratio)
    return np.float32(np.mean(loss, dtype=np.float32))


def _run_on_hw(in_maps):
    from concourse.bass_utils import run_bass_kernel_spmd
    nc = _get_program()
    try:
        res = run_bass_kernel_spmd(nc, in_maps, list(range(N_CORES)))
    except Exception:
        # transient NRT/axon failures: one retry
        res = run_bass_kernel_spmd(nc, in_maps, list(range(N_CORES)))
    return res.results


def kernel(feat1, feat2, indices1, indices2):
    in_maps = _prep_inputs(feat1, feat2, indices1, indices2)
    results = _run_on_hw(in_maps)
    return _assemble_loss(results)


# revision 21
# speedup vs baseline: 30.8050x; 29.7372x over previous
"""Trainium2 Bass kernel for nn_Neighbor_PairConLoss.

Computes, for pos = concat(feat1[i1], feat2[i1]) and neg = concat(feat1[i2], feat2[i2])
(both [8192, 256]):
    Ng = exp(pos @ neg.T / T).sum(-1);  Pg = exp(pos @ pos.T / T).sum(-1)
    loss = mean(-log(Pg / (Ng + Pg)))

Strategy: host-side gather + transpose (cheap, no HW time), then shard the 8192
rows of pos across 8 NeuronCores. Each core holds full negT/posT ([256, 8192],
feature-major) in SBUF plus its own 1024-row stationary slice, runs fp32r
matmuls into PSUM (N=512 per bank, groups of 4 banks), and fuses exp + row-sum
on the scalar engine via activation(Exp, scale=1/T, accum_out=...). Raw group
sums [128, 64] stream back per core; the final (tiny) reduction + log happens
on host in float32, matching the reference's IEEE semantics.
"""

import numpy as np

N_FEAT = 16384
D = 256
M_IDX = 4096
ROWS = 2 * M_IDX          # 8192 rows in pos/neg
INV_T = 20.0              # 1 / 0.05
N_CORES = 8
RPC = ROWS // N_CORES     # 1024 rows per core
RT = RPC // 128           # 8 row-tiles per core
KCH = D // 128            # 2 contraction chunks
CT = 512                  # matmul moving free dim (one PSUM bank)
GRP = 4 * CT              # 2048 columns per exp/accum group (4 banks)
NG = ROWS // GRP          # 4 groups per matrix per row-tile
# accumulator column layout: col = phase*32 + g*8 + m, plus 3 extra
# columns (64..66) holding the n=1..3 sub-sums of the split first group
# (phase=0, g=0, m=0, whose n=0 sub-sum lives in its normal col 0).
ACC_COLS = RT * 2 * NG + 3

_PROG = None  # cached (nc, meta)


def _build_program(repeats=1):
    import concourse.bass as bass
    import concourse.tile as tile
    from concourse import bacc, mybir

    f32 = mybir.dt.float32
    f32r = mybir.dt.float32r
    EXP = mybir.ActivationFunctionType.Exp

    nc = bacc.Bacc("TRN2", target_bir_lowering=False, debug=False)
    posT = nc.dram_tensor("posT", [KCH, 128, ROWS], f32r, kind="ExternalInput").ap()
    negT = nc.dram_tensor("negT", [KCH, 128, ROWS], f32r, kind="ExternalInput").ap()
    # my stationary slices, repacked host-side as [row-tile, partition, k*128+j]
    myT = nc.dram_tensor("myT", [RT, 128, KCH * 128], f32r,
                         kind="ExternalInput").ap()
    out = nc.dram_tensor("out", [128, ACC_COLS], f32, kind="ExternalOutput").ap()

    with tile.TileContext(nc) as tc:
        with tc.tile_pool(name="inp", bufs=1) as inp, \
             tc.tile_pool(name="ps", bufs=2, space="PSUM") as psp, \
             tc.tile_pool(name="scr", bufs=2) as scr, \
             tc.tile_pool(name="accp", bufs=1) as accp:

            # SBUF-resident inputs, chunked so matmuls only wait on the
            # chunks they read. DMA issue order = need order: the m=0
            # stationary slices and the first column-group gate the pipeline.
            my_t = [None] * RT
            neg_ch = [[None] * NG for _ in range(KCH)]
            pos_ch = [[None] * NG for _ in range(KCH)]

            def load_my(m):
                t = inp.tile([128, KCH * 128], f32r, tag=f"my_{m}")
                nc.sync.dma_start(t[:], myT[m])
                my_t[m] = t

            def my_ap(k, m):
                return my_t[m][:, k * 128:(k + 1) * 128]

            def load_grp(dst, src, g, name):
                for k in range(KCH):
                    t = inp.tile([128, GRP], f32r, tag=f"{name}{k}_{g}")
                    nc.sync.dma_start(t[:], src[k][:, g * GRP:(g + 1) * GRP])
                    dst[k][g] = t

            # First column-group of neg arrives in 512-col sub-chunks so the
            # pipeline starts as early as possible.
            neg_sub = [[None] * 4 for _ in range(KCH)]

            def load_neg0_sub(n):
                # n=0 dispatches ride the scalar engine's HWDGE queue so they
                # don't serialize behind sync's dispatches at t=0.
                eng = nc.scalar if n == 0 else nc.sync
                for k in range(KCH):
                    t = inp.tile([128, CT], f32r, tag=f"negs{k}_{n}")
                    eng.dma_start(t[:], negT[k][:, n * CT:(n + 1) * CT])
                    neg_sub[k][n] = t

            load_my(0)
            for n in range(4):
                load_neg0_sub(n)
            for m in range(1, RT):
                load_my(m)
            for g in range(1, NG):
                load_grp(neg_ch, negT, g, "neg")
            for g in range(NG):
                load_grp(pos_ch, posT, g, "pos")

            acc = accp.tile([128, ACC_COLS], f32)

            # Warm the ACT exp table (~2.7us) under the initial DMAs.
            warm = scr.tile([128, GRP], f32, tag="sc")
            nc.scalar.activation(warm[:, 0:1], warm[:, 1:2], EXP, scale=0.0)

            # phase 0: Ng (vs neg), phase 1: Pg (vs pos). Ng first so compute
            # can start while posT still streams in. g outer / m inner: the
            # first 2MB column-group feeds 8 row-tiles of compute, hiding the
            # DMA stream-in of later groups.
            def rhs_tile(phase, g, k, n):
                if phase == 0 and g == 0:
                    return neg_sub[k][n][:]
                X_ch = neg_ch if phase == 0 else pos_ch
                return X_ch[k][g][:, n * CT:(n + 1) * CT]

            for rep in range(repeats):
              for phase in (0, 1):
                for g in range(NG):
                    for m in range(RT):
                        col = phase * 32 + g * 8 + m
                        ps = psp.tile([128, GRP], f32, tag="ps")
                        first = (rep == 0 and phase == 0 and g == 0 and m == 0)
                        if first:
                            # Split the very first group per 512-col bank so
                            # exp starts as soon as the first chunks land.
                            for n in range(4):
                                for k in range(KCH):
                                    nc.tensor.matmul(
                                        ps[:, n * CT:(n + 1) * CT],
                                        my_ap(k, m),
                                        rhs_tile(phase, g, k, n),
                                        start=(k == 0),
                                        stop=(k == KCH - 1),
                                    )
                                sc = scr.tile([128, GRP], f32, tag="sc")
                                subcol = col if n == 0 else 63 + n
                                nc.scalar.activation(
                                    sc[:, 0:CT], ps[:, n * CT:(n + 1) * CT],
                                    EXP, scale=INV_T,
                                    accum_out=acc[:, subcol:subcol + 1],
                                )
                            continue
                        for k in range(KCH):
                            lhsT = my_ap(k, m)
                            for n in range(4):
                                nc.tensor.matmul(
                                    ps[:, n * CT:(n + 1) * CT],
                                    lhsT,
                                    rhs_tile(phase, g, k, n),
                                    start=(k == 0),
                                    stop=(k == KCH - 1),
                                )
                        sc = scr.tile([128, GRP], f32, tag="sc")
                        nc.scalar.activation(
                            sc[:], ps[:], EXP, scale=INV_T,
                            accum_out=acc[:, col:col + 1],
                        )
                if phase == 0:
                    # Ng columns are final: stream them out mid-kernel.
                    nc.sync.dma_start(out[:, 0:32], acc[:, 0:32])
                    nc.sync.dma_start(out[:, 64:ACC_COLS], acc[:, 64:ACC_COLS])
            nc.sync.dma_start(out[:, 32:64], acc[:, 32:64])

    nc.compile()
    return nc


def _get_program():
    global _PROG
    if _PROG is None:
        _PROG = _build_program()
    return _PROG


def _round_fp32r(x):
    """Round float32 to fp32r (1s + 8e + 11m, low 12 bits zero), nearest-even."""
    u = x.view(np.uint32)
    lsb = (u >> 12) & np.uint32(1)
    r = (u + np.uint32(0x7FF) + lsb) & np.uint32(0xFFFFF000)
    return r.view(np.float32)


def _prep_inputs(feat1, feat2, indices1, indices2):
    feat1 = np.asarray(feat1, dtype=np.float32)
    feat2 = np.asarray(feat2, dtype=np.float32)
    i1 = np.asarray(indices1).astype(np.int64)
    i2 = np.asarray(indices2).astype(np.int64)
    pos = np.concatenate([feat1[i1], feat2[i1]], axis=0)          # [8192, 256]
    neg = np.concatenate([feat1[i2], feat2[i2]], axis=0)
    posT = _round_fp32r(np.ascontiguousarray(pos.T)).reshape(KCH, 128, ROWS)
    negT = _round_fp32r(np.ascontiguousarray(neg.T)).reshape(KCH, 128, ROWS)
    in_maps = []
    for c in range(N_CORES):
        # myT[m, p, k*128+j] = posT[k, p, c*RPC + m*128 + j]
        sl = posT[:, :, c * RPC:(c + 1) * RPC].reshape(KCH, 128, RT, 128)
        myT = np.ascontiguousarray(sl.transpose(2, 1, 0, 3)).reshape(
            RT, 128, KCH * 128)
        in_maps.append({"posT": posT, "negT": negT, "myT": myT})
    return in_maps


def _assemble_loss(results):
    # results[c]["out"] is [128, 67]: col = phase*32 + g*8 + m, and cols
    # 64..66 hold the n=1..3 sub-sums of (phase0, g0, m0).
    ng = np.empty(ROWS, dtype=np.float32)
    pg = np.empty(ROWS, dtype=np.float32)
    for c in range(N_CORES):
        o = results[c]["out"]  # [128, ACC_COLS]
        for m in range(RT):
            base = c * RPC + m * 128
            cols = [g * 8 + m for g in range(NG)]
            ng_sum = o[:, cols].sum(axis=1, dtype=np.float32)
            if m == 0:
                ng_sum = (ng_sum + o[:, 64:67].sum(axis=1, dtype=np.float32)
                          ).astype(np.float32)
            ng[base:base + 128] = ng_sum
            pg[base:base + 128] = o[:, [32 + x for x in cols]].sum(
                axis=1, dtype=np.float32)
    with np.errstate(divide="ignore", invalid="ignore", over="ignore"):
        ratio = pg / (ng + pg)
        loss = -np.log(ratio)
    return np.float32(np.mean(loss, dtype=np.float32))


def _run_on_hw(in_maps):
    from concourse.bass_utils import run_bass_kernel_spmd
    nc = _get_program()
    try:
        res = run_bass_kernel_spmd(nc, in_maps, list(range(N_CORES)))
    except Exception:
        # transient NRT/axon failures: one retry
        res = run_bass_kernel_spmd(nc, in_maps, list(range(N_CORES)))
    return res.results


def kernel(feat1, feat2, indices1, indices2):
    in_maps = _prep_inputs(feat1, feat2, indices1, indices2)
    results = _run_on_hw(in_maps)
    return _assemble_loss(results)


# revision 24
# speedup vs baseline: 31.9771x; 1.0380x over previous
"""Trainium2 Bass kernel for nn_Neighbor_PairConLoss.

Computes, for pos = concat(feat1[i1], feat2[i1]) and neg = concat(feat1[i2], feat2[i2])
(both [8192, 256]):
    Ng = exp(pos @ neg.T / T).sum(-1);  Pg = exp(pos @ pos.T / T).sum(-1)
    loss = mean(-log(Pg / (Ng + Pg)))

Strategy: host-side gather + transpose + fp32r rounding (cheap, no HW time),
then shard the 8192 rows of pos across 8 NeuronCores. Each core holds full
negT/posT ([256, 8192], feature-major, fp32r) in SBUF plus its own 1024-row
stationary slices, runs fp32r matmuls into PSUM (N=512 per bank, groups of 4
banks, full-rate at 1 cyc/row), and fuses exp + row-sum on the scalar engine
via activation(Exp, scale=1/T, accum_out=...) reading/writing PSUM in place.
Raw group sums [128, 67] stream back per core; the final (tiny) reduction +
log happens on host in float32, matching the reference's IEEE semantics.
HW time ~143us (cost-model, validated against HW by R-repeat differencing).
"""

import numpy as np

N_FEAT = 16384
D = 256
M_IDX = 4096
ROWS = 2 * M_IDX          # 8192 rows in pos/neg
INV_T = 20.0              # 1 / 0.05
N_CORES = 8
RPC = ROWS // N_CORES     # 1024 rows per core
RT = RPC // 128           # 8 row-tiles per core
KCH = D // 128            # 2 contraction chunks
CT = 512                  # matmul moving free dim (one PSUM bank)
GRP = 4 * CT              # 2048 columns per exp/accum group (4 banks)
NG = ROWS // GRP          # 4 groups per matrix per row-tile
# accumulator column layout: col = phase*32 + g*8 + m, plus 3 extra
# columns (64..66) holding the n=1..3 sub-sums of the split first group
# (phase=0, g=0, m=0, whose n=0 sub-sum lives in its normal col 0).
ACC_COLS = RT * 2 * NG + 3

_PROG = None  # cached (nc, meta)


def _build_program(repeats=1):
    import concourse.bass as bass
    import concourse.tile as tile
    from concourse import bacc, mybir

    f32 = mybir.dt.float32
    f32r = mybir.dt.float32r
    EXP = mybir.ActivationFunctionType.Exp

    nc = bacc.Bacc("TRN2", target_bir_lowering=False, debug=False)
    posT = nc.dram_tensor("posT", [KCH, 128, ROWS], f32r, kind="ExternalInput").ap()
    negT = nc.dram_tensor("negT", [KCH, 128, ROWS], f32r, kind="ExternalInput").ap()
    # my stationary slices, repacked host-side as [row-tile, partition, k*128+j]
    myT = nc.dram_tensor("myT", [RT, 128, KCH * 128], f32r,
                         kind="ExternalInput").ap()
    out = nc.dram_tensor("out", [128, ACC_COLS], f32, kind="ExternalOutput").ap()

    with tile.TileContext(nc) as tc:
        with tc.tile_pool(name="inp", bufs=1) as inp, \
             tc.tile_pool(name="ps", bufs=2, space="PSUM") as psp, \
             tc.tile_pool(name="scr", bufs=2) as scr, \
             tc.tile_pool(name="accp", bufs=1) as accp:

            # SBUF-resident inputs, chunked so matmuls only wait on the
            # chunks they read. DMA issue order = need order: the m=0
            # stationary slices and the first column-group gate the pipeline.
            my_t = [None] * RT
            neg_ch = [[None] * NG for _ in range(KCH)]
            pos_ch = [[None] * NG for _ in range(KCH)]

            def load_my(m):
                t = inp.tile([128, KCH * 128], f32r, tag=f"my_{m}")
                nc.sync.dma_start(t[:], myT[m])
                my_t[m] = t

            def my_ap(k, m):
                return my_t[m][:, k * 128:(k + 1) * 128]

            def load_grp(dst, src, g, name):
                for k in range(KCH):
                    t = inp.tile([128, GRP], f32r, tag=f"{name}{k}_{g}")
                    nc.sync.dma_start(t[:], src[k][:, g * GRP:(g + 1) * GRP])
                    dst[k][g] = t

            # First column-group of neg arrives in 512-col sub-chunks so the
            # pipeline starts as early as possible.
            neg_sub = [[None] * 4 for _ in range(KCH)]

            def load_neg0_sub(n):
                # n=0 dispatches ride the scalar engine's HWDGE queue so they
                # don't serialize behind sync's dispatches at t=0.
                eng = nc.scalar if n == 0 else nc.sync
                for k in range(KCH):
                    t = inp.tile([128, CT], f32r, tag=f"negs{k}_{n}")
                    eng.dma_start(t[:], negT[k][:, n * CT:(n + 1) * CT])
                    neg_sub[k][n] = t

            load_my(0)
            for n in range(4):
                load_neg0_sub(n)
            for m in range(1, RT):
                load_my(m)
            for g in range(1, NG):
                load_grp(neg_ch, negT, g, "neg")
            for g in range(NG):
                load_grp(pos_ch, posT, g, "pos")

            acc = accp.tile([128, ACC_COLS], f32)

            # Warm the ACT exp table (~2.7us) under the initial DMAs.
            warm = scr.tile([128, 2], f32, tag="sc")
            nc.scalar.activation(warm[:, 0:1], warm[:, 1:2], EXP, scale=0.0)

            # phase 0: Ng (vs neg), phase 1: Pg (vs pos). Ng first so compute
            # can start while posT still streams in. g outer / m inner: the
            # first 2MB column-group feeds 8 row-tiles of compute, hiding the
            # DMA stream-in of later groups.
            def rhs_tile(phase, g, k, n):
                if phase == 0 and g == 0:
                    return neg_sub[k][n][:]
                X_ch = neg_ch if phase == 0 else pos_ch
                return X_ch[k][g][:, n * CT:(n + 1) * CT]

            for rep in range(repeats):
              for phase in (0, 1):
                for g in range(NG):
                    for m in range(RT):
                        col = phase * 32 + g * 8 + m
                        ps = psp.tile([128, GRP], f32, tag="ps")
                        first = (rep == 0 and phase == 0 and g == 0 and m == 0)
                        if first:
                            # Split the very first group per 512-col bank so
                            # exp starts as soon as the first chunks land.
                            for n in range(4):
                                for k in range(KCH):
                                    nc.tensor.matmul(
                                        ps[:, n * CT:(n + 1) * CT],
                                        my_ap(k, m),
                                        rhs_tile(phase, g, k, n),
                                        start=(k == 0),
                                        stop=(k == KCH - 1),
                                    )
                                subcol = col if n == 0 else 63 + n
                                nc.scalar.activation(
                                    ps[:, n * CT:(n + 1) * CT],
                                    ps[:, n * CT:(n + 1) * CT],
                                    EXP, scale=INV_T,
                                    accum_out=acc[:, subcol:subcol + 1],
                                )
                            continue
                        for k in range(KCH):
                            lhsT = my_ap(k, m)
                            for n in range(4):
                                nc.tensor.matmul(
                                    ps[:, n * CT:(n + 1) * CT],
                                    lhsT,
                                    rhs_tile(phase, g, k, n),
                                    start=(k == 0),
                                    stop=(k == KCH - 1),
                                )
                        nc.scalar.activation(
                            ps[:], ps[:], EXP, scale=INV_T,
                            accum_out=acc[:, col:col + 1],
                        )
                if phase == 0:
                    # Ng columns are final: stream them out mid-kernel.
                    nc.sync.dma_start(out[:, 0:32], acc[:, 0:32])
                    nc.sync.dma_start(out[:, 64:ACC_COLS], acc[:, 64:ACC_COLS])
            nc.sync.dma_start(out[:, 32:64], acc[:, 32:64])

    nc.compile()
    return nc


def _get_program():
    global _PROG
    if _PROG is None:
        _PROG = _build_program()
    return _PROG


def _round_fp32r(x):
    """Round float32 to fp32r (1s + 8e + 11m, low 12 bits zero), nearest-even."""
    u = x.view(np.uint32)
    lsb = (u >> 12) & np.uint32(1)
    r = (u + np.uint32(0x7FF) + lsb) & np.uint32(0xFFFFF000)
    return r.view(np.float32)


def _prep_inputs(feat1, feat2, indices1, indices2):
    feat1 = np.asarray(feat1, dtype=np.float32)
    feat2 = np.asarray(feat2, dtype=np.float32)
    i1 = np.asarray(indices1).astype(np.int64)
    i2 = np.asarray(indices2).astype(np.int64)
    pos = np.concatenate([feat1[i1], feat2[i1]], axis=0)          # [8192, 256]
    neg = np.concatenate([feat1[i2], feat2[i2]], axis=0)
    posT = _round_fp32r(np.ascontiguousarray(pos.T)).reshape(KCH, 128, ROWS)
    negT = _round_fp32r(np.ascontiguousarray(neg.T)).reshape(KCH, 128, ROWS)
    in_maps = []
    for c in range(N_CORES):
        # myT[m, p, k*128+j] = posT[k, p, c*RPC + m*128 + j]
        sl = posT[:, :, c * RPC:(c + 1) * RPC].reshape(KCH, 128, RT, 128)
        myT = np.ascontiguousarray(sl.transpose(2, 1, 0, 3)).reshape(
            RT, 128, KCH * 128)
        in_maps.append({"posT": posT, "negT": negT, "myT": myT})
    return in_maps


def _assemble_loss(results):
    # results[c]["out"] is [128, 67]: col = phase*32 + g*8 + m, and cols
    # 64..66 hold the n=1..3 sub-sums of (phase0, g0, m0).
    ng = np.empty(ROWS, dtype=np.float32)
    pg = np.empty(ROWS, dtype=np.float32)
    for c in range(N_CORES):
        o = results[c]["out"]  # [128, ACC_COLS]
        for m in range(RT):
            base = c * RPC + m * 128
            cols = [g * 8 + m for g in range(NG)]
            ng_sum = o[:, cols].sum(axis=1, dtype=np.float32)
            if m == 0:
                ng_sum = (ng_sum + o[:, 64:67].sum(axis=1, dtype=np.float32)
                          ).astype(np.float32)
            ng[base:base + 128] = ng_sum
            pg[base:base + 128] = o[:, [32 + x for x in cols]].sum(
                axis=1, dtype=np.float32)
    with np.errstate(divide="ignore", invalid="ignore", over="ignore"):
        ratio = pg / (ng + pg)
        loss = -np.log(ratio)
    return np.float32(np.mean(loss, dtype=np.float32))


def _run_on_hw(in_maps):
    from concourse.bass_utils import run_bass_kernel_spmd
    nc = _get_program()
    try:
        res = run_bass_kernel_spmd(nc, in_maps, list(range(N_CORES)))
    except Exception:
        # transient NRT/axon failures: one retry
        res = run_bass_kernel_spmd(nc, in_maps, list(range(N_CORES)))
    return res.results


def kernel(feat1, feat2, indices1, indices2):
    in_maps = _prep_inputs(feat1, feat2, indices1, indices2)
    results = _run_on_hw(in_maps)
    return _assemble_loss(results)


# revision 32
# speedup vs baseline: 49.4526x; 1.5465x over previous
"""Trainium2 Bass kernel for nn_Neighbor_PairConLoss.

Computes, for pos = concat(feat1[i1], feat2[i1]) and neg = concat(feat1[i2], feat2[i2])
(both [8192, 256]):
    Ng = exp(pos @ neg.T / T).sum(-1);  Pg = exp(pos @ pos.T / T).sum(-1)
    loss = mean(-log(Pg / (Ng + Pg)))

Strategy: host-side gather + transpose + fp32r rounding (cheap, no HW time),
then shard the 8192 rows of pos across 8 NeuronCores. Each core holds full
negT/posT ([256, 8192], feature-major, fp32r) in SBUF plus its own 1024-row
stationary slices, runs fp32r matmuls into PSUM (N=512 per bank, groups of 4
banks, full-rate at 1 cyc/row), and fuses exp + row-sum on the scalar engine
via activation(Exp, scale=1/T, accum_out=...) reading/writing PSUM in place.
Raw group sums [128, 67] stream back per core; the final (tiny) reduction +
log happens on host in float32, matching the reference's IEEE semantics.
HW time ~143us (cost-model, validated against HW by R-repeat differencing).
"""

import numpy as np

N_FEAT = 16384
D = 256
M_IDX = 4096
ROWS = 2 * M_IDX          # 8192 rows in pos/neg
INV_T = 20.0              # 1 / 0.05
N_CORES = 8
RPC = ROWS // N_CORES     # 1024 rows per core
RT = RPC // 128           # 8 row-tiles per core
KCH = D // 128            # 2 contraction chunks
CT = 512                  # matmul moving free dim (one PSUM bank)
GRP = 4 * CT              # 2048 columns per exp/accum group (4 banks)
NG = ROWS // GRP          # 4 groups per matrix per row-tile
# accumulator column layout: col = phase*32 + g*8 + m, plus 3 extra
# columns (64..66) holding the n=1..3 sub-sums of the split first group
# (phase=0, g=0, m=0, whose n=0 sub-sum lives in its normal col 0).
ACC_COLS = RT * 2 * NG + 3
# row-dedup mode: 57 distinct-row tiles = 7296 rows; 7 owned per core, the
# 57th tile's columns split 8 ways (1024 cols per core per phase, acc col 31).
D_TILES = 57
D_ROWS = D_TILES * 128
RT_OWN = 7

_PROGS = {}  # mode -> compiled program


def _build_program(repeats=1, dedup=False):
    import concourse.bass as bass
    import concourse.tile as tile
    from concourse import bacc, mybir

    f32 = mybir.dt.float32
    f32r = mybir.dt.float32r
    EXP = mybir.ActivationFunctionType.Exp

    nc = bacc.Bacc("TRN2", target_bir_lowering=False, debug=False)
    posT = nc.dram_tensor("posT", [KCH, 128, ROWS], f32r, kind="ExternalInput").ap()
    negT = nc.dram_tensor("negT", [KCH, 128, ROWS], f32r, kind="ExternalInput").ap()
    # my stationary slices, repacked host-side as [row-tile, partition, k*128+j]
    # (dedup mode: slots 0..6 = owned tiles, slot 7 = the shared 57th tile)
    myT = nc.dram_tensor("myT", [RT, 128, KCH * 128], f32r,
                         kind="ExternalInput").ap()
    if dedup:
        shNeg = nc.dram_tensor("shNeg", [KCH, 128, RPC], f32r,
                               kind="ExternalInput").ap()
        shPos = nc.dram_tensor("shPos", [KCH, 128, RPC], f32r,
                               kind="ExternalInput").ap()
    out = nc.dram_tensor("out", [128, ACC_COLS], f32, kind="ExternalOutput").ap()

    with tile.TileContext(nc) as tc:
        with tc.tile_pool(name="inp", bufs=1) as inp, \
             tc.tile_pool(name="ps", bufs=2, space="PSUM") as psp, \
             tc.tile_pool(name="scr", bufs=2) as scr, \
             tc.tile_pool(name="accp", bufs=1) as accp:

            # SBUF-resident inputs, chunked so matmuls only wait on the
            # chunks they read. DMA issue order = need order: the m=0
            # stationary slices and the first column-group gate the pipeline.
            my_t = [None] * RT
            neg_ch = [[None] * NG for _ in range(KCH)]
            pos_ch = [[None] * NG for _ in range(KCH)]

            def load_my(m):
                t = inp.tile([128, KCH * 128], f32r, tag=f"my_{m}")
                nc.sync.dma_start(t[:], myT[m])
                my_t[m] = t

            def my_ap(k, m):
                return my_t[m][:, k * 128:(k + 1) * 128]

            def load_grp(dst, src, g, name):
                for k in range(KCH):
                    t = inp.tile([128, GRP], f32r, tag=f"{name}{k}_{g}")
                    nc.sync.dma_start(t[:], src[k][:, g * GRP:(g + 1) * GRP])
                    dst[k][g] = t

            # First column-group of neg arrives in 512-col sub-chunks so the
            # pipeline starts as early as possible.
            neg_sub = [[None] * 4 for _ in range(KCH)]

            def load_neg0_sub(n):
                # n=0 dispatches ride the scalar engine's HWDGE queue so they
                # don't serialize behind sync's dispatches at t=0.
                eng = nc.scalar if n == 0 else nc.sync
                for k in range(KCH):
                    t = inp.tile([128, CT], f32r, tag=f"negs{k}_{n}")
                    eng.dma_start(t[:], negT[k][:, n * CT:(n + 1) * CT])
                    neg_sub[k][n] = t

            load_my(0)
            for n in range(4):
                load_neg0_sub(n)
            for m in range(1, RT):
                load_my(m)
            for g in range(1, NG):
                load_grp(neg_ch, negT, g, "neg")
            sh_t = {}
            if dedup:
                for nm, src in (("shNeg", shNeg), ("shPos", shPos)):
                    for k in range(KCH):
                        t = inp.tile([128, RPC], f32r, tag=f"{nm}{k}")
                        nc.sync.dma_start(t[:], src[k])
                        sh_t[(nm, k)] = t
            for g in range(NG):
                load_grp(pos_ch, posT, g, "pos")

            acc = accp.tile([128, ACC_COLS], f32)

            # Warm the ACT exp table (~2.7us) under the initial DMAs.
            warm = scr.tile([128, 2], f32, tag="sc")
            nc.scalar.activation(warm[:, 0:1], warm[:, 1:2], EXP, scale=0.0)

            # phase 0: Ng (vs neg), phase 1: Pg (vs pos). Ng first so compute
            # can start while posT still streams in. g outer / m inner: the
            # first 2MB column-group feeds 8 row-tiles of compute, hiding the
            # DMA stream-in of later groups.
            def rhs_tile(phase, g, k, n):
                if phase == 0 and g == 0:
                    return neg_sub[k][n][:]
                X_ch = neg_ch if phase == 0 else pos_ch
                return X_ch[k][g][:, n * CT:(n + 1) * CT]

            n_own = RT_OWN if dedup else RT
            for rep in range(repeats):
              for phase in (0, 1):
                for g in range(NG):
                    for m in range(n_own):
                        col = phase * 32 + g * 8 + m
                        ps = psp.tile([128, GRP], f32, tag="ps")
                        first = (rep == 0 and phase == 0 and g == 0 and m == 0)
                        if first:
                            # Split the very first group per 512-col bank so
                            # exp starts as soon as the first chunks land.
                            for n in range(4):
                                for k in range(KCH):
                                    nc.tensor.matmul(
                                        ps[:, n * CT:(n + 1) * CT],
                                        my_ap(k, m),
                                        rhs_tile(phase, g, k, n),
                                        start=(k == 0),
                                        stop=(k == KCH - 1),
                                    )
                                subcol = col if n == 0 else 63 + n
                                nc.scalar.activation(
                                    ps[:, n * CT:(n + 1) * CT],
                                    ps[:, n * CT:(n + 1) * CT],
                                    EXP, scale=INV_T,
                                    accum_out=acc[:, subcol:subcol + 1],
                                )
                            continue
                        for k in range(KCH):
                            lhsT = my_ap(k, m)
                            for n in range(4):
                                nc.tensor.matmul(
                                    ps[:, n * CT:(n + 1) * CT],
                                    lhsT,
                                    rhs_tile(phase, g, k, n),
                                    start=(k == 0),
                                    stop=(k == KCH - 1),
                                )
                        nc.scalar.activation(
                            ps[:], ps[:], EXP, scale=INV_T,
                            accum_out=acc[:, col:col + 1],
                        )
                if dedup:
                    # Shared 57th tile: this core's 1024-column slice (2
                    # banks), partial sum into acc col 31 of this phase.
                    sh_nm = "shNeg" if phase == 0 else "shPos"
                    ps = psp.tile([128, GRP], f32, tag="ps")
                    for k in range(KCH):
                        for n in range(2):
                            nc.tensor.matmul(
                                ps[:, n * CT:(n + 1) * CT],
                                my_ap(k, RT - 1),
                                sh_t[(sh_nm, k)][:, n * CT:(n + 1) * CT],
                                start=(k == 0),
                                stop=(k == KCH - 1),
                            )
                    shcol = phase * 32 + 31
                    nc.scalar.activation(
                        ps[:, 0:2 * CT], ps[:, 0:2 * CT], EXP, scale=INV_T,
                        accum_out=acc[:, shcol:shcol + 1],
                    )
                if phase == 0:
                    # Ng columns are final: stream them out mid-kernel.
                    nc.sync.dma_start(out[:, 0:32], acc[:, 0:32])
                    nc.sync.dma_start(out[:, 64:ACC_COLS], acc[:, 64:ACC_COLS])
            nc.sync.dma_start(out[:, 32:64], acc[:, 32:64])

    nc.compile()
    return nc


def _get_program(dedup=False):
    key = "dedup" if dedup else "dense"
    if key not in _PROGS:
        _PROGS[key] = _build_program(dedup=dedup)
    return _PROGS[key]


def _round_fp32r(x):
    """Round float32 to fp32r (1s + 8e + 11m, low 12 bits zero), nearest-even."""
    u = x.view(np.uint32)
    lsb = (u >> 12) & np.uint32(1)
    r = (u + np.uint32(0x7FF) + lsb) & np.uint32(0xFFFFF000)
    return r.view(np.float32)


def _repack_stationary(dT, tiles):
    """dT [KCH,128,R]; tiles: list of tile indices -> [len, 128, KCH*128]."""
    outp = np.empty((len(tiles), 128, KCH * 128), np.float32)
    for i, t in enumerate(tiles):
        for k in range(KCH):
            outp[i, :, k * 128:(k + 1) * 128] = dT[k, :, t * 128:(t + 1) * 128]
    return outp


def _prep_inputs(feat1, feat2, indices1, indices2, u1=None):
    """u1=None -> dense mode; else dedup mode with u1 = unique(indices1)."""
    feat1 = np.asarray(feat1, dtype=np.float32)
    feat2 = np.asarray(feat2, dtype=np.float32)
    i1 = np.asarray(indices1).astype(np.int64)
    i2 = np.asarray(indices2).astype(np.int64)
    pos = np.concatenate([feat1[i1], feat2[i1]], axis=0)          # [8192, 256]
    neg = np.concatenate([feat1[i2], feat2[i2]], axis=0)
    posT = _round_fp32r(np.ascontiguousarray(pos.T)).reshape(KCH, 128, ROWS)
    negT = _round_fp32r(np.ascontiguousarray(neg.T)).reshape(KCH, 128, ROWS)
    in_maps = []
    if u1 is None:
        for c in range(N_CORES):
            sl = posT[:, :, c * RPC:(c + 1) * RPC].reshape(KCH, 128, RT, 128)
            myT = np.ascontiguousarray(sl.transpose(2, 1, 0, 3)).reshape(
                RT, 128, KCH * 128)
            in_maps.append({"posT": posT, "negT": negT, "myT": myT})
        return in_maps
    # dedup: distinct pos rows, zero-padded to D_ROWS, feature-major
    pos_d = np.concatenate([feat1[u1], feat2[u1]], axis=0)
    pos_d = np.pad(pos_d, ((0, D_ROWS - pos_d.shape[0]), (0, 0)))
    dT = _round_fp32r(np.ascontiguousarray(pos_d.T)).reshape(KCH, 128, D_ROWS)
    sh_my = _repack_stationary(dT, [D_TILES - 1])[0]
    for c in range(N_CORES):
        myT = np.empty((RT, 128, KCH * 128), np.float32)
        myT[:RT_OWN] = _repack_stationary(
            dT, list(range(c * RT_OWN, (c + 1) * RT_OWN)))
        myT[RT - 1] = sh_my
        in_maps.append({
            "posT": posT, "negT": negT, "myT": myT,
            "shNeg": np.ascontiguousarray(negT[:, :, c * RPC:(c + 1) * RPC]),
            "shPos": np.ascontiguousarray(posT[:, :, c * RPC:(c + 1) * RPC]),
        })
    return in_maps


def _finish(ng, pg):
    with np.errstate(divide="ignore", invalid="ignore", over="ignore"):
        loss = -np.log(pg / (ng + pg))
    return np.float32(np.mean(loss, dtype=np.float32))


def _sum_cols(o, m, phase):
    cols = [phase * 32 + g * 8 + m for g in range(NG)]
    s = o[:, cols].sum(axis=1, dtype=np.float32)
    if phase == 0 and m == 0:
        s = (s + o[:, 64:67].sum(axis=1, dtype=np.float32)).astype(np.float32)
    return s


def _assemble_dense(results):
    # col = phase*32 + g*8 + m; cols 64..66: n=1..3 sub-sums of (ph0,g0,m0)
    ng = np.empty(ROWS, dtype=np.float32)
    pg = np.empty(ROWS, dtype=np.float32)
    for c in range(N_CORES):
        o = results[c]["out"]
        for m in range(RT):
            base = c * RPC + m * 128
            ng[base:base + 128] = _sum_cols(o, m, 0)
            pg[base:base + 128] = _sum_cols(o, m, 1)
    return _finish(ng, pg)


def _assemble_dedup(results, i1, u1):
    ngd = np.empty(D_ROWS, dtype=np.float32)
    pgd = np.empty(D_ROWS, dtype=np.float32)
    for c in range(N_CORES):
        o = results[c]["out"]
        for m in range(RT_OWN):
            base = (c * RT_OWN + m) * 128
            ngd[base:base + 128] = _sum_cols(o, m, 0)
            pgd[base:base + 128] = _sum_cols(o, m, 1)
    base = (D_TILES - 1) * 128
    ngd[base:base + 128] = sum(
        results[c]["out"][:, 31] for c in range(N_CORES)).astype(np.float32)
    pgd[base:base + 128] = sum(
        results[c]["out"][:, 63] for c in range(N_CORES)).astype(np.float32)
    # expand distinct-row values back to the original 8192 rows
    n_u = len(u1)
    idx = np.searchsorted(u1, i1)                 # position of each index in u1
    full = np.concatenate([idx, n_u + idx])       # f1-half then f2-half
    return _finish(ngd[full], pgd[full])


def _run_on_hw(in_maps, dedup):
    from concourse.bass_utils import run_bass_kernel_spmd
    nc = _get_program(dedup)
    try:
        res = run_bass_kernel_spmd(nc, in_maps, list(range(N_CORES)))
    except Exception:
        # transient NRT/axon failures: one retry
        res = run_bass_kernel_spmd(nc, in_maps, list(range(N_CORES)))
    return res.results


def kernel(feat1, feat2, indices1, indices2):
    i1 = np.asarray(indices1).astype(np.int64)
    u1 = np.unique(i1)
    if 2 * len(u1) <= D_ROWS:
        in_maps = _prep_inputs(feat1, feat2, indices1, indices2, u1=u1)
        results = _run_on_hw(in_maps, dedup=True)
        return _assemble_dedup(results, i1, u1)
    in_maps = _prep_inputs(feat1, feat2, indices1, indices2)
    results = _run_on_hw(in_maps, dedup=False)
    return _assemble_dense(results)
